# revision 9
# baseline (speedup 1.0000x reference)
"""CapsuleLayer Bass/Tile kernel for TRN2 (one NeuronCore; replicated SPMD x8).

Host-side prep transposes+casts x to fp16 in a per-b-tile layout so the
device does no transposes at all:
  xin[t*128 + p, j*128 + b] = x[t*128 + b, j*128 + p]   (fp16)
Per 128-sample b-tile t, the 16 chunks j are matmul lhsT operands
directly: out[b, c] += sum_p xin_t[p, j*128+b] * kpad[p, j*PADN+c].

kpad (fp16) holds the [2048, 160] kernel as [128, 16*PADN] with col
160:176 of each chunk = 0.1 * sum over capsule blocks, so the first
routing iteration's weighted sum falls out of the matmul.

Routing runs on [128, G*160] group-wide tiles (DVE/ACT/GPSIMD). sqrt is
computed as exp(0.5*ln(x)) so every ACT func (Copy/Ln/Exp) lives in one
activation table set (no LoadActFuncSet thrash).
"""

from dataclasses import dataclass

import numpy as np

import concourse.bacc as bacc
import concourse.tile as tile
from concourse import mybir

NCAP = 10
DCAP = 16
EPS = 1e-7
D = 2048
NCOL = NCAP * DCAP  # 160
NCHUNK = 16  # 2048 / 128


@dataclass
class Cfg:
    n_btiles: int = 16          # 128-sample tiles per core
    group: int = 4              # b-tiles per routing group
    pad_n: int = 176            # kpad columns per chunk (160 hat + 16 s1)
    n_cores: int = 8
    reps: int = 1               # repeat whole pipeline (for slope timing)
    dma_btiles: int = 1         # b-tiles per input DMA
    loop_reps: int = 0          # >0: wrap body in a hardware For_i loop
    group_sizes: str = ""       # e.g. "8,5,3"; overrides group when set
    big_pat: str = "ddpdddpd"   # engine per big routing op (p=Pool, d=DVE)
    pump: int = 8               # routing steps pumped per b-tile emitted
    x_bufs: int = 6
    phat_bufs: int = 8

    @property
    def bc(self):
        return self.n_btiles * 128


def prep_x(x_core: np.ndarray, n_btiles: int) -> np.ndarray:
    """[Bc, 2048] f32 -> [Bc, 2048] fp16 laid out [t*128+p, j*128+b]."""
    bc = x_core.shape[0]
    assert x_core.shape == (bc, D) and bc == n_btiles * 128
    a = x_core.reshape(n_btiles, 128, NCHUNK, 128)  # [t, b, j, p]
    a = np.ascontiguousarray(a.transpose(0, 3, 2, 1))  # [t, p, j, b]
    return a.reshape(bc, D).astype(np.float16)


def make_kpad(kernel: np.ndarray, pad_n: int) -> np.ndarray:
    """[2048, 160] f32 -> [128, 16*pad_n] fp16, kpad[p, j*pad_n+c] =
    k[j*128+p, c]; col 160:176 = 0.1 * sum over capsules; rest zero."""
    d, ncol = kernel.shape
    assert (d, ncol) == (D, NCOL)
    kp = np.zeros((NCHUNK, 128, pad_n), dtype=np.float32)
    kj = kernel.reshape(NCHUNK, 128, NCOL)
    kp[:, :, :NCOL] = kj
    kp[:, :, NCOL:NCOL + DCAP] = 0.1 * kj.reshape(NCHUNK, 128, NCAP, DCAP).sum(axis=2)
    out = kp.transpose(1, 0, 2).reshape(128, NCHUNK * pad_n)
    return np.ascontiguousarray(out).astype(np.float16)


def build(cfg: Cfg):
    nc = bacc.Bacc("TRN2", target_bir_lowering=False, debug=False,
                   num_devices=cfg.n_cores)
    f16 = mybir.dt.float16
    f32 = mybir.dt.float32

    NB = cfg.n_btiles
    PADN = cfg.pad_n
    if cfg.group_sizes:
        sizes = [int(s) for s in cfg.group_sizes.split(",")]
    else:
        assert NB % cfg.group == 0
        sizes = [cfg.group] * (NB // cfg.group)
    assert sum(sizes) == NB

    eps_t = nc.alloc_sbuf_tensor("const-eps", [128, 1], f32)
    nc.gpsimd.memset(eps_t.ap(), EPS)
    nc.const_aps.aps[(f32, EPS)] = eps_t.ap()
    nc.all_engine_barrier()

    xin = nc.dram_tensor("xin", [cfg.bc, D], f16, kind="ExternalInput")
    kpad = nc.dram_tensor("kpad", [128, NCHUNK * PADN], f16,
                          kind="ExternalInput")
    # tiny input consumed by a scratch DMA: lets the bench chain iterations
    # device-side (seed <- slice of yout) to time the NEFF without host RTT
    seed = nc.dram_tensor("seed", [128, DCAP], f32, kind="ExternalInput")
    yout = nc.dram_tensor("yout", [cfg.bc, DCAP], f32, kind="ExternalOutput")

    with tile.TileContext(nc) as tc:
        with (
            tc.tile_pool(name="const", bufs=1) as constp,
            tc.tile_pool(name="xt", bufs=cfg.x_bufs) as xtp,
            tc.tile_pool(name="phat", bufs=cfg.phat_bufs, space="PSUM") as phatp,
            tc.tile_pool(name="hatw", bufs=3) as hatwp,
            tc.tile_pool(name="rt", bufs=3) as rtp,
            tc.tile_pool(name="sm", bufs=3) as smp,
            tc.tile_pool(name="outs", bufs=3) as outsp,
        ):
            seed_t = constp.tile([128, DCAP], f32, tag="seed")
            nc.sync.dma_start(seed_t[:], seed[:, :])
            kp_t = constp.tile([128, NCHUNK * PADN], f16, tag="kpad")

            xv = xin[:, :].rearrange("(t p) d -> t p d", p=128)

            DB = cfg.dma_btiles
            x_slices = {}  # i -> (tile, col offset)
            kpad_loaded = [False]

            def load_chunk(i0):
                xt = xtp.tile([128, DB * D], f16, tag="xt")
                if DB == 1:
                    nc.sync.dma_start(xt[:], xv[i0])
                else:
                    nc.sync.dma_start(
                        xt[:].rearrange("p (t d) -> p t d", t=DB),
                        xin[:, :].rearrange("(c t p) d -> c p t d",
                                            t=DB, p=128)[i0 // DB],
                    )
                for t in range(DB):
                    x_slices[i0 + t] = (xt, t * D)
                if not kpad_loaded[0]:
                    kpad_loaded[0] = True
                    nc.sync.dma_start(kp_t[:], kpad[:, :])

            S1C = PADN  # 176 = 11 * 16 per-tile block in hatw

            def big_engines():
                """Engine pattern for the 8 big [128, G*160] ops per group:
                order of use: mul2,red2, mulw2,redw2, mul3,red3, mulw3,redw3."""
                pat = []
                for ch in cfg.big_pat:
                    pat.append(nc.gpsimd if ch == "p" else nc.vector)
                return pat

            def routing_gen(i0, G, gi, hatw):
                """Generator emitting routing ops for one group; yields
                between ops so the driver can interleave groups (wavefront
                order keeps every engine queue head runnable)."""
                yv = yout[i0 * 128:(i0 + G) * 128, :].rearrange(
                    "(g p) d -> p g d", p=128)
                W = hatw[:].rearrange("p (g n d) -> p g n d", n=NCAP + 1,
                                      d=DCAP)
                H = W[:, :, 0:NCAP, :]
                S1 = W[:, :, NCAP, :]
                eng = big_engines()

                def sqrt_eps(n2, tag):
                    lg = smp.tile([128, G], f32, tag=f"lg{tag}")
                    nc.scalar.activation(lg[:], n2,
                                         mybir.ActivationFunctionType.Ln,
                                         bias=EPS)
                    sr = smp.tile([128, G], f32, tag=f"sr{tag}")
                    nc.scalar.activation(sr[:], lg[:],
                                         mybir.ActivationFunctionType.Exp,
                                         scale=0.5)
                    return sr

                def squash_steps(su, sdt, r, tag, out):
                    """out[0] = comb scale c s.t. v = c*su*(r or 1)."""
                    sq = smp.tile([128, G * DCAP], sdt, tag=f"sq{tag}")
                    nc.scalar.activation(sq[:], su,
                                         mybir.ActivationFunctionType.Square)
                    yield
                    m2 = smp.tile([128, G], f32, tag=f"m2{tag}")
                    nc.vector.tensor_reduce(
                        m2[:], sq[:].rearrange("p (g d) -> p g d", g=G),
                        axis=mybir.AxisListType.X, op=mybir.AluOpType.add)
                    yield
                    if r is not None:
                        rr = smp.tile([128, G], f32, tag=f"rr{tag}")
                        nc.vector.tensor_mul(rr[:], r, r)
                        n2 = smp.tile([128, G], f32, tag=f"n2{tag}")
                        nc.vector.tensor_mul(n2[:], m2[:], rr[:])
                        yield
                    else:
                        n2 = m2
                    sr = sqrt_eps(n2[:], tag)
                    yield
                    den = smp.tile([128, G], f32, tag=f"den{tag}")
                    nc.vector.scalar_tensor_tensor(
                        den[:], n2[:], 1.0, sr[:],
                        op0=mybir.AluOpType.add, op1=mybir.AluOpType.mult)
                    rec = smp.tile([128, G], f32, tag=f"rec{tag}")
                    nc.vector.reciprocal(rec[:], den[:])
                    yield
                    sc = smp.tile([128, G], f32, tag=f"sc{tag}")
                    nc.vector.tensor_mul(sc[:], n2[:], rec[:])
                    out[1] = sc
                    if r is not None:
                        comb = smp.tile([128, G], f32, tag=f"comb{tag}")
                        nc.vector.tensor_mul(comb[:], sc[:], r)
                        out[0] = comb
                    else:
                        out[0] = sc
                    yield

                def dots_steps(src_gd, e0, e1, tag, out):
                    """r[g,n] = sum_d H[g,n,d]*src[g,d] -> [128, G*NCAP] f32"""
                    tmp = rtp.tile([128, G * NCOL], f16, tag=f"dt{tag}")
                    bc = src_gd.unsqueeze(2).broadcast_to((128, G, NCAP, DCAP))
                    e0.tensor_mul(
                        tmp[:].rearrange("p (g n d) -> p g n d", g=G, n=NCAP),
                        H, bc)
                    yield
                    o = rtp.tile([128, G * NCAP], f32, tag=f"dr{tag}")
                    e1.tensor_reduce(
                        o[:], tmp[:].rearrange("p (g n d) -> p g n d",
                                               g=G, n=NCAP),
                        axis=mybir.AxisListType.X, op=mybir.AluOpType.add)
                    out[0] = o
                    yield

                def wsum_steps(e_gn, edt, e0, e1, tag, out):
                    """su[g,d] = sum_n H[g,n,d]*e[g,n] -> [128, G*DCAP] f32"""
                    tmp = rtp.tile([128, G * NCOL], edt, tag=f"wt{tag}")
                    bc = e_gn.unsqueeze(3).broadcast_to((128, G, NCAP, DCAP))
                    e0.tensor_mul(
                        tmp[:].rearrange("p (g n d) -> p g n d", g=G, n=NCAP),
                        H, bc)
                    yield
                    o = rtp.tile([128, G * DCAP], f32, tag=f"ws{tag}")
                    e1.tensor_reduce(
                        o[:], tmp[:].rearrange("p (g n d) -> p g d n",
                                               g=G, n=NCAP),
                        axis=mybir.AxisListType.X, op=mybir.AluOpType.add)
                    out[0] = o
                    yield

                def softmax_steps(t_gn, edt, tag, out):
                    """e = exp(t) [128, G*NCAP]; r = 1/sum_n e [128, G]"""
                    e = rtp.tile([128, G * NCAP], edt, tag=f"e{tag}")
                    nc.scalar.activation(e[:], t_gn,
                                         mybir.ActivationFunctionType.Exp)
                    yield
                    se = smp.tile([128, G], f32, tag=f"se{tag}")
                    nc.vector.tensor_reduce(
                        se[:], e[:].rearrange("p (g n) -> p g n", g=G),
                        axis=mybir.AxisListType.X, op=mybir.AluOpType.add)
                    ri = smp.tile([128, G], f32, tag=f"ri{tag}")
                    nc.vector.reciprocal(ri[:], se[:])
                    out[0] = (e, ri)
                    yield

                gv = lambda ap: ap.rearrange("p (g d) -> p g d", g=G)
                nv = lambda ap: ap.rearrange("p (g n) -> p g n", g=G)

                # iter 1: s1 (pre-scaled mean) came from the matmul (fp16)
                c1, r2o = [None, None], [None]
                yield from squash_steps(S1, f16, None, "1", c1)
                yield from dots_steps(S1, eng[0], eng[1], "2", r2o)
                t2 = rtp.tile([128, G * NCAP], f32, tag="t2")
                nc.gpsimd.tensor_mul(
                    nv(t2[:]), nv(r2o[0][:]),
                    c1[0][:].unsqueeze(2).broadcast_to((128, G, NCAP)))
                yield

                # iter 2
                sm2 = [None]
                yield from softmax_steps(t2[:], f16, "2", sm2)
                e2, r2i = sm2[0]
                s2o = [None]
                yield from wsum_steps(nv(e2[:]), f16, eng[2], eng[3], "2", s2o)
                s2u = s2o[0]
                s2h = rtp.tile([128, G * DCAP], f16, tag="s2h")
                nc.gpsimd.tensor_mul(
                    gv(s2h[:]), gv(s2u[:]),
                    r2i[:].unsqueeze(2).broadcast_to((128, G, DCAP)))
                yield
                c2, r3o = [None, None], [None]
                yield from squash_steps(s2u[:], f32, r2i[:], "2", c2)
                yield from dots_steps(gv(s2h[:]), eng[4], eng[5], "3", r3o)
                t3 = rtp.tile([128, G * NCAP], f32, tag="t3")
                nc.gpsimd.tensor_mul(
                    nv(t3[:]), nv(r3o[0][:]),
                    c2[1][:].unsqueeze(2).broadcast_to((128, G, NCAP)))
                nc.gpsimd.tensor_add(t3[:], t3[:], t2[:])
                yield

                # iter 3
                sm3 = [None]
                yield from softmax_steps(t3[:], f32, "3", sm3)
                e3, r3i = sm3[0]
                s3o = [None]
                yield from wsum_steps(nv(e3[:]), f32, eng[6], eng[7], "3", s3o)
                s3u = s3o[0]
                c3 = [None, None]
                yield from squash_steps(s3u[:], f32, r3i[:], "3", c3)
                v3 = outsp.tile([128, G * DCAP], f32, tag="v3")
                nc.vector.tensor_mul(
                    gv(v3[:]), gv(s3u[:]),
                    c3[0][:].unsqueeze(2).broadcast_to((128, G, DCAP)))
                nc.sync.dma_start(
                    yv,
                    v3[:].rearrange("p (g d) -> p g d", g=G))
                yield

            def run_all():
                active = []

                def pump(k):
                    for _ in range(k):
                        for gen in list(active):
                            try:
                                next(gen)
                            except StopIteration:
                                active.remove(gen)

                i0 = 0
                for gi, G in enumerate(sizes):
                    hatw = hatwp.tile([128, G * S1C], f16, tag="hatw")
                    for g in range(G):
                        i = i0 + g
                        if i % DB == 0:
                            load_chunk(i)
                        xt, off = x_slices.pop(i)
                        ph = phatp.tile([128, PADN], f32, tag="phat")
                        for j in range(NCHUNK):
                            nc.tensor.matmul(
                                ph[:],
                                xt[:, off + j * 128:off + (j + 1) * 128],
                                kp_t[:, j * PADN:(j + 1) * PADN],
                                start=(j == 0),
                                stop=(j == NCHUNK - 1),
                            )
                        nc.scalar.copy(hatw[:, g * S1C:(g + 1) * S1C],
                                       ph[:, :S1C])
                        pump(cfg.pump)
                    active.append(routing_gen(i0, G, gi, hatw))
                    i0 += G
                while active:
                    pump(1)

            if cfg.loop_reps > 0:
                with tc.For_i(0, cfg.loop_reps, 1,
                              hint_engines=(mybir.EngineType.PE,)):
                    run_all()
            else:
                for _rep in range(cfg.reps):
                    run_all()

    nc.compile()
    _unify_act_tables(nc)
    return nc


def _unify_act_tables(nc):
    """Replace the greedy per-func LoadActFuncSet placement with a single
    load of a set containing every activation func this kernel uses
    (Copy/Identity/Square/Exp/Ln all live in natural_log_exp_and_others).
    The greedy pass alternates exp_and_others <-> natural_log per squash,
    costing ~1.3us per reload on the ACT engine."""
    from concourse.hw_specs import get_activation_tables

    AF = mybir.ActivationFunctionType
    needed = {AF.Copy, AF.Identity, AF.Square, AF.Exp, AF.Ln}
    tables = list(get_activation_tables(nc.m.arch).items())
    combined_id = None
    for idx, (name, funcs) in enumerate(tables):
        if needed <= funcs:
            combined_id = idx
            break
    if combined_id is None:
        return  # no single set covers us; keep the pass's placement

    fn = nc.m.functions[0]
    blocks = list(fn.blocks)
    kept = None
    for blk in blocks:
        insts = list(blk.instructions)
        out = []
        for i in insts:
            if isinstance(i, mybir.InstLoadActFuncSet):
                if kept is None:
                    i.act_func_set_id = combined_id
                    kept = i
                continue  # drop (re-inserted once below)
            out.append(i)
        if len(out) != len(insts):
            blk.instructions[:] = out
    if kept is None:
        return
    # Place the single load in the entry block (before its terminator) so
    # it dominates every activation, including those inside For_i loops.
    entry = blocks[0]
    insts = list(entry.instructions)
    pos = len(insts)
    while pos > 0 and insts[pos - 1].opcode in ("UnconditionalBranch",
                                                "ConditionalBranch"):
        pos -= 1
    entry.instructions[:] = insts[:pos] + [kept] + insts[pos:]


# ---------------- numpy reference (per-core) ----------------

def ref_numpy(x: np.ndarray, kernel: np.ndarray) -> np.ndarray:
    b = x.shape[0]
    hat = (x @ kernel).reshape(b, NCAP, DCAP)
    logits = np.zeros((b, NCAP, 1), dtype=x.dtype)
    out = None
    for _ in range(3):
        ex = np.exp(logits - logits.max(axis=1, keepdims=True))
        c = ex / ex.sum(axis=1, keepdims=True)
        s = (c * hat).sum(axis=1, keepdims=True)
        s2 = np.square(s).sum(axis=-1, keepdims=True)
        out = s2 / (1.0 + s2) / np.sqrt(s2 + EPS) * s
        logits = logits + np.einsum("bnd,bd->bn", hat, out[:, 0, :])[:, :, None]
    return out[:, 0, :]


# ---------------- public entry point ----------------

_CACHE = {}

BEST = Cfg(n_btiles=16, group_sizes="5,5,4,2", pump=10)


def prep_in_maps(x: np.ndarray, kern: np.ndarray, cfg: Cfg):
    kpad = make_kpad(np.asarray(kern, dtype=np.float32), cfg.pad_n)
    seed = np.zeros((128, DCAP), dtype=np.float32)
    return [
        {"xin": prep_x(x[i * cfg.bc:(i + 1) * cfg.bc], cfg.n_btiles),
         "kpad": kpad, "seed": seed}
        for i in range(cfg.n_cores)
    ]


def kernel(inputs: np.ndarray, kernel: np.ndarray) -> np.ndarray:
    """CapsuleLayer forward: inputs [16384, 2048] f32, kernel [2048, 160] f32
    -> [16384, 16] f32. Runs SPMD across 8 NeuronCores (batch split 8 ways)."""
    from concourse.bass_utils import run_bass_kernel_spmd

    cfg = BEST
    assert inputs.shape == (cfg.bc * cfg.n_cores, D)
    assert kernel.shape == (D, NCOL)
    if "nc" not in _CACHE:
        _CACHE["nc"] = build(cfg)
    nc = _CACHE["nc"]

    x = np.ascontiguousarray(inputs, dtype=np.float32)
    in_maps = prep_in_maps(x, kernel, cfg)
    res = run_bass_kernel_spmd(nc, in_maps, list(range(cfg.n_cores)))
    return np.concatenate(
        [res.results[i]["yout"] for i in range(cfg.n_cores)], axis=0)
